# revision 7
# baseline (speedup 1.0000x reference)
"""ArcFace loss on 8 TRN2 NeuronCores (vocab/tensor-parallel over classes).

Math (per reference):
    cos = normalize(emb) @ normalize(W).T            [B, C]
    phi at label column only (ArcFace margin), logits = 64 * modified cos
    loss = mean CE(logits, labels)

Never materializes phi/sine over [B, C]: the margin only matters at the
label position.  Each core owns a contiguous class shard (transposed +
bf16 on host, zero-padded), computes partial row sums of
exp(64*cos - 64) on device (flash-style constant bias; valid because
max logit << 64), corrects its 1/8 share of the label column, and a
single small AllReduce combines the per-row sums.  The label term and
final scalar are computed redundantly on every core in f32.
"""

import math
import numpy as np
import ml_dtypes

import concourse.bass as bass
import concourse.mybir as mybir
from concourse import bacc, tile, masks
from concourse.bass_utils import run_bass_kernel_spmd

N_CORES = 8
B = 1024
D = 512
C = 100000
C_PER = C // N_CORES          # 12500
CP = 12800                    # per-core classes padded to 25 * 512
CB = 512                      # matmul free-dim block (one PSUM bank)
SUPER_CB = 3                  # class blocks per exp super-block (3 banks)
SCALE = 64.0
MARGIN = 0.5
EXP_BIAS = -16.0
EPS = 1e-12

FP32 = mybir.dt.float32
BF16 = mybir.dt.bfloat16
AF = mybir.ActivationFunctionType
ALU = mybir.AluOpType
X = mybir.AxisListType.X

COS_M = math.cos(MARGIN)
SIN_M = math.sin(MARGIN)
TH = math.cos(math.pi - MARGIN)
MM = math.sin(math.pi - MARGIN) * MARGIN


def _supers(n_blocks: int, super_cb: int):
    """[(first_block, n_cb), ...] covering n_blocks class blocks."""
    out = []
    b = 0
    while b < n_blocks:
        n = min(super_cb, n_blocks - b)
        out.append((b, n))
        b += n
    return out


def build_graph(b=B, cp=CP, super_cb=SUPER_CB):
    m_tiles = b // 128
    k_chunks = D // 128
    n_blocks = cp // CB
    supers = _supers(n_blocks, super_cb)
    n_sup = len(supers)

    nc = bacc.Bacc("TRN2", target_bir_lowering=False, debug=False,
                   num_devices=N_CORES)
    emb = nc.dram_tensor("emb", [b, D], FP32, kind="ExternalInput")
    wt = nc.dram_tensor("wt", [D, cp], BF16, kind="ExternalInput")
    wlab = nc.dram_tensor("wlab", [b, D], FP32, kind="ExternalInput")
    out = nc.dram_tensor("out", [1, 1], FP32, kind="ExternalOutput")

    emb_ap = emb.ap()
    wt_ap = wt.ap()
    wlab_ap = wlab.ap()

    with tile.TileContext(nc) as tc:
        with (
            tc.tile_pool(name="const", bufs=1) as cpool,
            tc.tile_pool(name="persist", bufs=1) as pp,
            tc.tile_pool(name="small", bufs=4) as sp,
            tc.tile_pool(name="scr", bufs=3) as scr,
        ):
            ident = cpool.tile([128, 128], BF16, tag="ident")
            masks.make_identity(nc, ident[:])
            ones_b = cpool.tile([128, 128], BF16, tag="ones_b")
            nc.vector.memset(ones_b[:], 1.0)
            ones_f = cpool.tile([128, 1], FP32, tag="ones_f")
            nc.vector.memset(ones_f[:], 1.0)
            bias_n = cpool.tile([128, 1], FP32, tag="bias_n")
            nc.vector.memset(bias_n[:], EXP_BIAS)
            bias_p = cpool.tile([128, 1], FP32, tag="bias_p")
            nc.vector.memset(bias_p[:], -EXP_BIAS)

            # persistent state
            ehat = [pp.tile([128, D], FP32, tag=f"ehat{m}", name=f"ehat{m}")
                    for m in range(m_tiles)]
            ehT = [pp.tile([128, b], BF16, tag=f"ehT{k}", name=f"ehT{k}")
                   for k in range(k_chunks)]
            acc = [pp.tile([128, n_sup], FP32, tag=f"acc{m}", name=f"acc{m}")
                   for m in range(m_tiles)]
            cosl = pp.tile([128, m_tiles], FP32, tag="cosl")
            tlab = pp.tile([128, m_tiles], FP32, tag="tlab")
            delta = pp.tile([128, m_tiles], FP32, tag="delta")
            sloc = pp.tile([128, m_tiles], FP32, tag="sloc")

            # ---- phase 0: embedding normalization + transpose ----
            with tc.tile_pool(name="psum_t", bufs=4, space="PSUM") as pst:
                for m in range(m_tiles):
                    et = scr.tile([128, D], FP32, tag="et")
                    nc.sync.dma_start(et[:], emb_ap[m * 128:(m + 1) * 128, :])
                    sq_s = scr.tile([128, D], FP32, tag="sq_s")
                    ssq = sp.tile([128, 1], FP32, tag="ssq")
                    nc.scalar.activation(sq_s[:], et[:], AF.Square, accum_out=ssq[:])
                    ssqc = sp.tile([128, 1], FP32, tag="ssqc")
                    nc.vector.tensor_scalar_max(ssqc[:], ssq[:], EPS * EPS)
                    lnq = sp.tile([128, 1], FP32, tag="lnq")
                    nc.scalar.activation(lnq[:], ssqc[:], AF.Ln)
                    re = sp.tile([128, 1], FP32, tag="re")
                    nc.scalar.activation(re[:], lnq[:], AF.Exp, scale=-0.5)
                    nc.vector.tensor_scalar_mul(ehat[m][:], et[:], re[:])
                    eb = scr.tile([128, D], BF16, tag="eb")
                    nc.vector.tensor_copy(eb[:], ehat[m][:])
                    for k in range(k_chunks):
                        pt = pst.tile([128, 128], BF16, tag="pt")
                        nc.tensor.transpose(pt[:], eb[:, k * 128:(k + 1) * 128],
                                            ident[:])
                        nc.vector.tensor_copy(
                            ehT[k][:, m * 128:(m + 1) * 128], pt[:])

            # ---- phase 1: streamed classes: norm, matmul, exp-accum ----
            with (
                tc.tile_pool(name="psum_g", bufs=2, space="PSUM") as psg,
                tc.tile_pool(name="psum_s", bufs=2, space="PSUM") as pss,
                tc.tile_pool(name="wpool", bufs=2 * k_chunks) as wp,
                tc.tile_pool(name="whpool", bufs=1) as whp,
                tc.tile_pool(name="sqpool", bufs=3) as sqp,
                tc.tile_pool(name="rwpool", bufs=2) as rwp,
                tc.tile_pool(name="expool", bufs=2) as exp_p,
            ):
                # double-buffered normalized-transposed weight tiles
                wh = {}
                for j in range(2):
                    for cb in range(super_cb):
                        for k in range(k_chunks):
                            wh[(j, cb, k)] = whp.tile([128, CB], BF16, tag=f"wh{j}_{cb}_{k}",
                                                      name=f"wh{j}_{cb}_{k}")
                for si, (cb0, ncb) in enumerate(supers):
                    j = si % 2
                    for cb in range(ncb):
                        co = (cb0 + cb) * CB
                        wts = []
                        ps = pss.tile([128, CB], FP32, tag="ps")
                        for k in range(k_chunks):
                            wtk = wp.tile([128, CB], BF16, tag="wt")
                            nc.sync.dma_start(
                                wtk[:], wt_ap[k * 128:(k + 1) * 128, co:co + CB])
                            wts.append(wtk)
                            sqk = sqp.tile([128, CB], BF16, tag="sq")
                            nc.vector.tensor_tensor(sqk[:], wtk[:], wtk[:],
                                                    ALU.mult)
                            nc.tensor.matmul(ps[:], ones_b[:], sqk[:],
                                             start=(k == 0),
                                             stop=(k == k_chunks - 1))
                        t0 = rwp.tile([128, CB], FP32, tag="t0")
                        nc.vector.tensor_scalar_max(t0[:], ps[:], 1e-24)
                        t1 = rwp.tile([128, CB], FP32, tag="t1")
                        nc.scalar.activation(t1[:], t0[:], AF.Ln)
                        rw = rwp.tile([128, CB], BF16, tag="rw")
                        nc.scalar.activation(rw[:], t1[:], AF.Exp, scale=-0.5)
                        for k in range(k_chunks):
                            nc.vector.tensor_tensor(wh[(j, cb, k)][:], wts[k][:],
                                                    rw[:], ALU.mult)
                    for m in range(m_tiles):
                        pg = psg.tile([128, super_cb * CB], FP32, tag="pg")
                        for cb in range(ncb):
                            for k in range(k_chunks):
                                nc.tensor.matmul(
                                    pg[:, cb * CB:(cb + 1) * CB],
                                    ehT[k][:, m * 128:(m + 1) * 128],
                                    wh[(j, cb, k)][:],
                                    start=(k == 0), stop=(k == k_chunks - 1))
                        ex = exp_p.tile([128, super_cb * CB], BF16, tag="ex")
                        nc.scalar.activation(
                            ex[:, :ncb * CB], pg[:, :ncb * CB], AF.Exp,
                            bias=bias_n[:], scale=SCALE,
                            accum_out=acc[m][:, si:si + 1])

            # ---- phase 2: label column correction (f32, replicated) ----
            for m in range(m_tiles):
                wlt = scr.tile([128, D], FP32, tag="wlt")
                nc.sync.dma_start(wlt[:], wlab_ap[m * 128:(m + 1) * 128, :])
                sq_s = scr.tile([128, D], FP32, tag="sq_s")
                ssql = sp.tile([128, 1], FP32, tag="ssql")
                nc.scalar.activation(sq_s[:], wlt[:], AF.Square,
                                     accum_out=ssql[:])
                ssqlc = sp.tile([128, 1], FP32, tag="ssqlc")
                nc.vector.tensor_scalar_max(ssqlc[:], ssql[:], EPS * EPS)
                lnl = sp.tile([128, 1], FP32, tag="lnl")
                nc.scalar.activation(lnl[:], ssqlc[:], AF.Ln)
                rl = sp.tile([128, 1], FP32, tag="rl")
                nc.scalar.activation(rl[:], lnl[:], AF.Exp, scale=-0.5)
                dsc = scr.tile([128, D], FP32, tag="dsc")
                dotm = sp.tile([128, 1], FP32, tag="dotm")
                nc.vector.tensor_tensor(dsc[:], ehat[m][:], wlt[:], ALU.mult)
                nc.vector.tensor_reduce(dotm[:], dsc[:], X, ALU.add)
                nc.vector.tensor_tensor(cosl[:, m:m + 1], dotm[:], rl[:],
                                        ALU.mult)

            c2 = sp.tile([128, m_tiles], FP32, tag="c2")
            nc.scalar.activation(c2[:], cosl[:], AF.Square)
            ss = sp.tile([128, m_tiles], FP32, tag="ss")
            nc.vector.tensor_scalar(ss[:], c2[:], -1.0, 1.0, ALU.mult, ALU.add)
            nc.vector.tensor_scalar_max(ss[:], ss[:], 1e-30)
            lns = sp.tile([128, m_tiles], FP32, tag="lns")
            nc.scalar.activation(lns[:], ss[:], AF.Ln)
            sinl = sp.tile([128, m_tiles], FP32, tag="sinl")
            nc.scalar.activation(sinl[:], lns[:], AF.Exp, scale=0.5)
            pa = sp.tile([128, m_tiles], FP32, tag="pa")
            nc.vector.tensor_scalar_mul(pa[:], cosl[:], COS_M)
            pb = sp.tile([128, m_tiles], FP32, tag="pb")
            nc.vector.tensor_scalar_mul(pb[:], sinl[:], SIN_M)
            phi = sp.tile([128, m_tiles], FP32, tag="phi")
            nc.vector.tensor_tensor(phi[:], pa[:], pb[:], ALU.subtract)
            alt = sp.tile([128, m_tiles], FP32, tag="alt")
            nc.vector.tensor_scalar_sub(alt[:], cosl[:], MM)
            msk = sp.tile([128, m_tiles], FP32, tag="msk")
            nc.vector.tensor_scalar(msk[:], cosl[:], TH, None, ALU.is_gt)
            dphi = sp.tile([128, m_tiles], FP32, tag="dphi")
            nc.vector.tensor_tensor(dphi[:], phi[:], alt[:], ALU.subtract)
            mphi = sp.tile([128, m_tiles], FP32, tag="mphi")
            nc.vector.tensor_tensor(mphi[:], msk[:], dphi[:], ALU.mult)
            phis = sp.tile([128, m_tiles], FP32, tag="phis")
            nc.vector.tensor_tensor(phis[:], alt[:], mphi[:], ALU.add)
            nc.vector.tensor_scalar_mul(tlab[:], phis[:], SCALE)
            ea = sp.tile([128, m_tiles], FP32, tag="ea")
            nc.scalar.activation(ea[:], phis[:], AF.Exp, bias=bias_n[:],
                                 scale=SCALE)
            eb2 = sp.tile([128, m_tiles], FP32, tag="eb2")
            nc.scalar.activation(eb2[:], cosl[:], AF.Exp, bias=bias_n[:],
                                 scale=SCALE)
            dl = sp.tile([128, m_tiles], FP32, tag="dl")
            nc.vector.tensor_tensor(dl[:], ea[:], eb2[:], ALU.subtract)
            nc.vector.tensor_scalar_mul(delta[:], dl[:], 1.0 / N_CORES)

            # ---- phase 3: reduce, all-reduce, final scalar ----
            for m in range(m_tiles):
                nc.vector.tensor_reduce(sloc[:, m:m + 1], acc[m][:], X, ALU.add)
            scorr = sp.tile([128, m_tiles], FP32, tag="scorr")
            nc.vector.tensor_tensor(scorr[:], sloc[:], delta[:], ALU.add)

            with (
                tc.tile_pool(name="dram", bufs=2, space="DRAM") as dp,
                tc.tile_pool(name="psum_f", bufs=1, space="PSUM") as psf,
            ):
                in_b = dp.tile([128, m_tiles], FP32)
                out_b = dp.tile([128, m_tiles], FP32)
                nc.sync.dma_start(in_b[:], scorr[:])
                nc.gpsimd.collective_compute(
                    "AllReduce", ALU.add,
                    replica_groups=[list(range(N_CORES))],
                    ins=[in_b.opt()], outs=[out_b.opt()])
                sg = sp.tile([128, m_tiles], FP32, tag="sg")
                nc.sync.dma_start(sg[:], out_b[:])
                logs = sp.tile([128, m_tiles], FP32, tag="logs")
                nc.scalar.activation(logs[:], sg[:], AF.Ln)
                ld = sp.tile([128, m_tiles], FP32, tag="ld")
                nc.vector.tensor_tensor(ld[:], logs[:], tlab[:], ALU.subtract)
                lp = sp.tile([128, 1], FP32, tag="lp")
                nc.vector.tensor_reduce(lp[:], ld[:], X, ALU.add)
                pf = psf.tile([1, 1], FP32)
                nc.tensor.matmul(pf[:], ones_f[:], lp[:])
                fin = sp.tile([1, 1], FP32, tag="fin")
                nc.scalar.activation(fin[:], pf[:], AF.Identity,
                                     scale=1.0 / b, bias=bias_p[:1])
                nc.sync.dma_start(out.ap()[:, :], fin[:])

    nc.compile()
    return nc


def make_in_maps(embeddings, weight, labels, b=B, cp=CP):
    emb = np.ascontiguousarray(np.asarray(embeddings, np.float32))
    w = np.asarray(weight, np.float32)
    lab = np.asarray(labels).astype(np.int64)
    c, d = w.shape
    c_per = c // N_CORES
    wlab = np.ascontiguousarray(w[lab])
    wT = w.T  # [D, C]
    in_maps = []
    for i in range(N_CORES):
        wt_i = np.zeros((d, cp), ml_dtypes.bfloat16)
        wt_i[:, :c_per] = wT[:, i * c_per:(i + 1) * c_per].astype(
            ml_dtypes.bfloat16)
        in_maps.append({"emb": emb, "wt": wt_i, "wlab": wlab})
    return in_maps


_CACHED_NC = None


def kernel(embeddings, weight, labels):
    global _CACHED_NC
    if _CACHED_NC is None:
        _CACHED_NC = build_graph()
    in_maps = make_in_maps(embeddings, weight, labels)
    res = run_bass_kernel_spmd(_CACHED_NC, in_maps,
                               core_ids=list(range(N_CORES)), trace=False)
    val = np.asarray(res.results[0]["out"], np.float32).reshape(())
    return val


if __name__ == "__main__":
    rng = np.random.default_rng(0)
    e = rng.standard_normal((B, D)).astype(np.float32)
    w = (rng.random((C, D), np.float32) - 0.5) * 0.015
    l = rng.integers(0, C, B).astype(np.int64)
    print(kernel(e, w, l))


# revision 8
# speedup vs baseline: 1.2175x; 1.2175x over previous
"""ArcFace loss on 8 TRN2 NeuronCores (vocab/tensor-parallel over classes).

Math (per reference):
    cos = normalize(emb) @ normalize(W).T            [B, C]
    phi at label column only (ArcFace margin), logits = 64 * modified cos
    loss = mean CE(logits, labels)

Never materializes phi/sine over [B, C]: the margin only matters at the
label position.  Each core owns a contiguous class shard (transposed +
bf16 on host, zero-padded), computes partial row sums of
exp(64*cos - 64) on device (flash-style constant bias; valid because
max logit << 64), corrects its 1/8 share of the label column, and a
single small AllReduce combines the per-row sums.  The label term and
final scalar are computed redundantly on every core in f32.
"""

import math
import numpy as np
import ml_dtypes

import concourse.bass as bass
import concourse.mybir as mybir
from concourse import bacc, tile, masks
from concourse.bass_utils import run_bass_kernel_spmd

# Pin every ACT instruction to the one table set that covers all functions
# this kernel uses (exp, ln, square, identity, copy) so the activation
# table is loaded once instead of thrashing between per-function sets.
_ACT_SET = "natural_log_exp_and_others"
_orig_get_act_tables = bacc.get_activation_tables


def _pinned_act_tables(arch):
    tables = _orig_get_act_tables(arch)
    if _ACT_SET in tables:
        return {name: (fns if name == _ACT_SET else set())
                for name, fns in tables.items()}
    return tables


bacc.get_activation_tables = _pinned_act_tables

N_CORES = 8
B = 1024
D = 512
C = 100000
C_PER = C // N_CORES          # 12500
CP = 12800                    # per-core classes padded to 25 * 512
CB = 512                      # matmul free-dim block (one PSUM bank)
SUPER_CB = 3                  # class blocks per exp super-block (3 banks)
SCALE = 64.0
MARGIN = 0.5
EXP_BIAS = -16.0
EPS = 1e-12

FP32 = mybir.dt.float32
BF16 = mybir.dt.bfloat16
AF = mybir.ActivationFunctionType
ALU = mybir.AluOpType
X = mybir.AxisListType.X

COS_M = math.cos(MARGIN)
SIN_M = math.sin(MARGIN)
TH = math.cos(math.pi - MARGIN)
MM = math.sin(math.pi - MARGIN) * MARGIN


def _supers(n_blocks: int, super_cb: int):
    """[(first_block, n_cb), ...] covering n_blocks class blocks."""
    out = []
    b = 0
    while b < n_blocks:
        n = min(super_cb, n_blocks - b)
        out.append((b, n))
        b += n
    return out


def build_graph(b=B, cp=CP, super_cb=SUPER_CB):
    m_tiles = b // 128
    k_chunks = D // 128
    n_blocks = cp // CB
    supers = _supers(n_blocks, super_cb)
    n_sup = len(supers)

    nc = bacc.Bacc("TRN2", target_bir_lowering=False, debug=False,
                   num_devices=N_CORES)
    emb = nc.dram_tensor("emb", [b, D], FP32, kind="ExternalInput")
    wt = nc.dram_tensor("wt", [D, cp], BF16, kind="ExternalInput")
    wlab = nc.dram_tensor("wlab", [b, D], FP32, kind="ExternalInput")
    out = nc.dram_tensor("out", [1, 1], FP32, kind="ExternalOutput")

    emb_ap = emb.ap()
    wt_ap = wt.ap()
    wlab_ap = wlab.ap()

    with tile.TileContext(nc) as tc:
        with (
            tc.tile_pool(name="const", bufs=1) as cpool,
            tc.tile_pool(name="persist", bufs=1) as pp,
            tc.tile_pool(name="small", bufs=4) as sp,
            tc.tile_pool(name="scr", bufs=3) as scr,
        ):
            ident = cpool.tile([128, 128], BF16, tag="ident")
            masks.make_identity(nc, ident[:])
            ones_b = cpool.tile([128, 128], BF16, tag="ones_b")
            nc.vector.memset(ones_b[:], 1.0)
            ones_f = cpool.tile([128, 1], FP32, tag="ones_f")
            nc.vector.memset(ones_f[:], 1.0)
            bias_n = cpool.tile([128, 1], FP32, tag="bias_n")
            nc.vector.memset(bias_n[:], EXP_BIAS)
            bias_p = cpool.tile([128, 1], FP32, tag="bias_p")
            nc.vector.memset(bias_p[:], -EXP_BIAS)

            # persistent state
            ehat = [pp.tile([128, D], FP32, tag=f"ehat{m}", name=f"ehat{m}")
                    for m in range(m_tiles)]
            ehT = [pp.tile([128, b], BF16, tag=f"ehT{k}", name=f"ehT{k}")
                   for k in range(k_chunks)]
            acc = [pp.tile([128, n_sup], FP32, tag=f"acc{m}", name=f"acc{m}")
                   for m in range(m_tiles)]
            cosl = pp.tile([128, m_tiles], FP32, tag="cosl")
            tlab = pp.tile([128, m_tiles], FP32, tag="tlab")
            delta = pp.tile([128, m_tiles], FP32, tag="delta")
            sloc = pp.tile([128, m_tiles], FP32, tag="sloc")

            # ---- phase 0: embedding normalization + transpose ----
            with tc.tile_pool(name="psum_t", bufs=4, space="PSUM") as pst:
                for m in range(m_tiles):
                    et = scr.tile([128, D], FP32, tag="et")
                    nc.sync.dma_start(et[:], emb_ap[m * 128:(m + 1) * 128, :])
                    sq_s = scr.tile([128, D], FP32, tag="sq_s")
                    ssq = sp.tile([128, 1], FP32, tag="ssq")
                    nc.scalar.activation(sq_s[:], et[:], AF.Square, accum_out=ssq[:])
                    ssqc = sp.tile([128, 1], FP32, tag="ssqc")
                    nc.vector.tensor_scalar_max(ssqc[:], ssq[:], EPS * EPS)
                    lnq = sp.tile([128, 1], FP32, tag="lnq")
                    nc.scalar.activation(lnq[:], ssqc[:], AF.Ln)
                    re = sp.tile([128, 1], FP32, tag="re")
                    nc.scalar.activation(re[:], lnq[:], AF.Exp, scale=-0.5)
                    nc.vector.tensor_scalar_mul(ehat[m][:], et[:], re[:])
                    eb = scr.tile([128, D], BF16, tag="eb")
                    nc.vector.tensor_copy(eb[:], ehat[m][:])
                    for k in range(k_chunks):
                        pt = pst.tile([128, 128], BF16, tag="pt")
                        nc.tensor.transpose(pt[:], eb[:, k * 128:(k + 1) * 128],
                                            ident[:])
                        nc.vector.tensor_copy(
                            ehT[k][:, m * 128:(m + 1) * 128], pt[:])

            # ---- phase 1: streamed classes: norm, matmul, exp-accum ----
            with (
                tc.tile_pool(name="psum_g", bufs=2, space="PSUM") as psg,
                tc.tile_pool(name="psum_s", bufs=2, space="PSUM") as pss,
                tc.tile_pool(name="wpool", bufs=2 * k_chunks) as wp,
                tc.tile_pool(name="whpool", bufs=1) as whp,
                tc.tile_pool(name="sqpool", bufs=3) as sqp,
                tc.tile_pool(name="rwpool", bufs=2) as rwp,
                tc.tile_pool(name="expool", bufs=2) as exp_p,
            ):
                # double-buffered normalized-transposed weight tiles
                wh = {}
                for j in range(2):
                    for cb in range(super_cb):
                        for k in range(k_chunks):
                            wh[(j, cb, k)] = whp.tile([128, CB], BF16, tag=f"wh{j}_{cb}_{k}",
                                                      name=f"wh{j}_{cb}_{k}")
                for si, (cb0, ncb) in enumerate(supers):
                    j = si % 2
                    for cb in range(ncb):
                        co = (cb0 + cb) * CB
                        wts = []
                        ps = pss.tile([128, CB], FP32, tag="ps")
                        for k in range(k_chunks):
                            wtk = wp.tile([128, CB], BF16, tag="wt")
                            nc.sync.dma_start(
                                wtk[:], wt_ap[k * 128:(k + 1) * 128, co:co + CB])
                            wts.append(wtk)
                            sqk = sqp.tile([128, CB], BF16, tag="sq")
                            nc.vector.tensor_tensor(sqk[:], wtk[:], wtk[:],
                                                    ALU.mult)
                            nc.tensor.matmul(ps[:], ones_b[:], sqk[:],
                                             start=(k == 0),
                                             stop=(k == k_chunks - 1))
                        t0 = rwp.tile([128, CB], FP32, tag="t0")
                        nc.vector.tensor_scalar_max(t0[:], ps[:], 1e-24)
                        t1 = rwp.tile([128, CB], FP32, tag="t1")
                        nc.scalar.activation(t1[:], t0[:], AF.Ln)
                        rw = rwp.tile([128, CB], BF16, tag="rw")
                        nc.scalar.activation(rw[:], t1[:], AF.Exp, scale=-0.5)
                        for k in range(k_chunks):
                            nc.vector.tensor_tensor(wh[(j, cb, k)][:], wts[k][:],
                                                    rw[:], ALU.mult)
                    for m in range(m_tiles):
                        pg = psg.tile([128, super_cb * CB], FP32, tag="pg")
                        for cb in range(ncb):
                            for k in range(k_chunks):
                                nc.tensor.matmul(
                                    pg[:, cb * CB:(cb + 1) * CB],
                                    ehT[k][:, m * 128:(m + 1) * 128],
                                    wh[(j, cb, k)][:],
                                    start=(k == 0), stop=(k == k_chunks - 1))
                        ex = exp_p.tile([128, super_cb * CB], BF16, tag="ex")
                        nc.scalar.activation(
                            ex[:, :ncb * CB], pg[:, :ncb * CB], AF.Exp,
                            bias=bias_n[:], scale=SCALE,
                            accum_out=acc[m][:, si:si + 1])

            # ---- phase 2: label column correction (f32, replicated) ----
            for m in range(m_tiles):
                wlt = scr.tile([128, D], FP32, tag="wlt")
                nc.sync.dma_start(wlt[:], wlab_ap[m * 128:(m + 1) * 128, :])
                sq_s = scr.tile([128, D], FP32, tag="sq_s")
                ssql = sp.tile([128, 1], FP32, tag="ssql")
                nc.scalar.activation(sq_s[:], wlt[:], AF.Square,
                                     accum_out=ssql[:])
                ssqlc = sp.tile([128, 1], FP32, tag="ssqlc")
                nc.vector.tensor_scalar_max(ssqlc[:], ssql[:], EPS * EPS)
                lnl = sp.tile([128, 1], FP32, tag="lnl")
                nc.scalar.activation(lnl[:], ssqlc[:], AF.Ln)
                rl = sp.tile([128, 1], FP32, tag="rl")
                nc.scalar.activation(rl[:], lnl[:], AF.Exp, scale=-0.5)
                dsc = scr.tile([128, D], FP32, tag="dsc")
                dotm = sp.tile([128, 1], FP32, tag="dotm")
                nc.vector.tensor_tensor(dsc[:], ehat[m][:], wlt[:], ALU.mult)
                nc.vector.tensor_reduce(dotm[:], dsc[:], X, ALU.add)
                nc.vector.tensor_tensor(cosl[:, m:m + 1], dotm[:], rl[:],
                                        ALU.mult)

            c2 = sp.tile([128, m_tiles], FP32, tag="c2")
            nc.scalar.activation(c2[:], cosl[:], AF.Square)
            ss = sp.tile([128, m_tiles], FP32, tag="ss")
            nc.vector.tensor_scalar(ss[:], c2[:], -1.0, 1.0, ALU.mult, ALU.add)
            nc.vector.tensor_scalar_max(ss[:], ss[:], 1e-30)
            lns = sp.tile([128, m_tiles], FP32, tag="lns")
            nc.scalar.activation(lns[:], ss[:], AF.Ln)
            sinl = sp.tile([128, m_tiles], FP32, tag="sinl")
            nc.scalar.activation(sinl[:], lns[:], AF.Exp, scale=0.5)
            pa = sp.tile([128, m_tiles], FP32, tag="pa")
            nc.vector.tensor_scalar_mul(pa[:], cosl[:], COS_M)
            pb = sp.tile([128, m_tiles], FP32, tag="pb")
            nc.vector.tensor_scalar_mul(pb[:], sinl[:], SIN_M)
            phi = sp.tile([128, m_tiles], FP32, tag="phi")
            nc.vector.tensor_tensor(phi[:], pa[:], pb[:], ALU.subtract)
            alt = sp.tile([128, m_tiles], FP32, tag="alt")
            nc.vector.tensor_scalar_sub(alt[:], cosl[:], MM)
            msk = sp.tile([128, m_tiles], FP32, tag="msk")
            nc.vector.tensor_scalar(msk[:], cosl[:], TH, None, ALU.is_gt)
            dphi = sp.tile([128, m_tiles], FP32, tag="dphi")
            nc.vector.tensor_tensor(dphi[:], phi[:], alt[:], ALU.subtract)
            mphi = sp.tile([128, m_tiles], FP32, tag="mphi")
            nc.vector.tensor_tensor(mphi[:], msk[:], dphi[:], ALU.mult)
            phis = sp.tile([128, m_tiles], FP32, tag="phis")
            nc.vector.tensor_tensor(phis[:], alt[:], mphi[:], ALU.add)
            nc.vector.tensor_scalar_mul(tlab[:], phis[:], SCALE)
            ea = sp.tile([128, m_tiles], FP32, tag="ea")
            nc.scalar.activation(ea[:], phis[:], AF.Exp, bias=bias_n[:],
                                 scale=SCALE)
            eb2 = sp.tile([128, m_tiles], FP32, tag="eb2")
            nc.scalar.activation(eb2[:], cosl[:], AF.Exp, bias=bias_n[:],
                                 scale=SCALE)
            dl = sp.tile([128, m_tiles], FP32, tag="dl")
            nc.vector.tensor_tensor(dl[:], ea[:], eb2[:], ALU.subtract)
            nc.vector.tensor_scalar_mul(delta[:], dl[:], 1.0 / N_CORES)

            # ---- phase 3: reduce, all-reduce, final scalar ----
            for m in range(m_tiles):
                nc.vector.tensor_reduce(sloc[:, m:m + 1], acc[m][:], X, ALU.add)
            scorr = sp.tile([128, m_tiles], FP32, tag="scorr")
            nc.vector.tensor_tensor(scorr[:], sloc[:], delta[:], ALU.add)

            with (
                tc.tile_pool(name="dram", bufs=2, space="DRAM") as dp,
                tc.tile_pool(name="psum_f", bufs=1, space="PSUM") as psf,
            ):
                in_b = dp.tile([128, m_tiles], FP32)
                out_b = dp.tile([128, m_tiles], FP32)
                nc.sync.dma_start(in_b[:], scorr[:])
                nc.gpsimd.collective_compute(
                    "AllReduce", ALU.add,
                    replica_groups=[list(range(N_CORES))],
                    ins=[in_b.opt()], outs=[out_b.opt()])
                sg = sp.tile([128, m_tiles], FP32, tag="sg")
                nc.sync.dma_start(sg[:], out_b[:])
                logs = sp.tile([128, m_tiles], FP32, tag="logs")
                nc.scalar.activation(logs[:], sg[:], AF.Ln)
                ld = sp.tile([128, m_tiles], FP32, tag="ld")
                nc.vector.tensor_tensor(ld[:], logs[:], tlab[:], ALU.subtract)
                lp = sp.tile([128, 1], FP32, tag="lp")
                nc.vector.tensor_reduce(lp[:], ld[:], X, ALU.add)
                pf = psf.tile([1, 1], FP32)
                nc.tensor.matmul(pf[:], ones_f[:], lp[:])
                fin = sp.tile([1, 1], FP32, tag="fin")
                nc.scalar.activation(fin[:], pf[:], AF.Identity,
                                     scale=1.0 / b, bias=bias_p[:1])
                nc.sync.dma_start(out.ap()[:, :], fin[:])

    nc.compile()
    return nc


def make_in_maps(embeddings, weight, labels, b=B, cp=CP):
    emb = np.ascontiguousarray(np.asarray(embeddings, np.float32))
    w = np.asarray(weight, np.float32)
    lab = np.asarray(labels).astype(np.int64)
    c, d = w.shape
    c_per = c // N_CORES
    wlab = np.ascontiguousarray(w[lab])
    wT = w.T  # [D, C]
    in_maps = []
    for i in range(N_CORES):
        wt_i = np.zeros((d, cp), ml_dtypes.bfloat16)
        wt_i[:, :c_per] = wT[:, i * c_per:(i + 1) * c_per].astype(
            ml_dtypes.bfloat16)
        in_maps.append({"emb": emb, "wt": wt_i, "wlab": wlab})
    return in_maps


_CACHED_NC = None


def kernel(embeddings, weight, labels):
    global _CACHED_NC
    if _CACHED_NC is None:
        _CACHED_NC = build_graph()
    in_maps = make_in_maps(embeddings, weight, labels)
    res = run_bass_kernel_spmd(_CACHED_NC, in_maps,
                               core_ids=list(range(N_CORES)), trace=False)
    val = np.asarray(res.results[0]["out"], np.float32).reshape(())
    return val


if __name__ == "__main__":
    rng = np.random.default_rng(0)
    e = rng.standard_normal((B, D)).astype(np.float32)
    w = (rng.random((C, D), np.float32) - 0.5) * 0.015
    l = rng.integers(0, C, B).astype(np.int64)
    print(kernel(e, w, l))
